# revision 32
# baseline (speedup 1.0000x reference)
"""CPAttention Trainium2 kernel: 8-way batch-data-parallel over 8 NeuronCores.

v3: fp16 hi/lo split matmuls for the argsort-critical score path.
  - qkproj: x,w split on HOST into fp16 (hi, lo); q = xh*wh + xh*wl + xl*wh
    (12 fp16 matmuls per output tile, fp32 PSUM accumulate, err ~2^-24)
  - dots: q,k split on DEVICE into fp16 hi/lo (Scalar: hi cast, Vector: lo sub);
    d = hiK*hiQ + loK*hiQ + hiK*loQ  (6 K=64 fp16 matmuls per tile, row-packed
    head pairs at tile_position (0,0)/(64,0))
  - score pack rows accumulate across ALL prs in one persistent PSUM tile:
    row0 = score_A, row64 = score_B (fp32); Z_A row32 / Z_B row96 per pr (bf16)
  - nnz, score scaling, bias add, argsort + 16-step row swap: on host
  - output projection DMAs straight from PSUM (no bias add on device)
Softmax/output path is bf16 (rel-err budget 2e-2); score path is fp32-class.
"""
import numpy as np

import concourse.bacc as bacc
import concourse.tile as tile
from concourse import mybir
from concourse.bass_utils import run_bass_kernel_spmd

F32 = mybir.dt.float32
F32R = mybir.dt.float32r
F16 = mybir.dt.float16
BF16 = mybir.dt.bfloat16
U32 = mybir.dt.uint32
AOP = mybir.AluOpType
AFT = mybir.ActivationFunctionType

B, N, DIM = 8, 1024, 512
HEADS, DH = 8, 64
INNER = 512
SCALE = DH ** -0.5

_cache = {}


def _emit_burst(nc, oTp, pack, vv, ones32, onesbf, pr, jt, es, abs_):
    first, last = (jt == 0), (jt == 7)
    for hh in range(2):
        nc.tensor.matmul(
            oTp[hh * 64:(hh + 1) * 64, :],
            vv[:, 2 * pr + hh, jt, :], es[hh][:, :],
            start=first, stop=last,
            tile_position=(0, hh * 64),
            skip_group_check=True)
    for ic in range(2):
        sl = slice(ic * 512, (ic + 1) * 512)
        for hh in range(2):
            st = hh * 64
            nc.tensor.matmul(
                pack[st:st + 1, sl],
                ones32, abs_[hh][:, sl],
                start=first, stop=last,
                tile_position=(0, st),
                skip_group_check=True)
    for hh in range(2):
        st = 32 + hh * 64
        nc.tensor.matmul(
            pack[st:st + 1, :],
            onesbf, es[hh][:, :],
            start=first, stop=last,
            tile_position=(0, st),
            skip_group_check=True)


def _build():
    nc = bacc.Bacc()
    xh_d = nc.declare_dram_parameter("xh", [DIM, N], F16, isOutput=False)
    xl_d = nc.declare_dram_parameter("xl", [DIM, N], F16, isOutput=False)
    xTbf = nc.declare_dram_parameter("xTbf", [DIM, N], BF16, isOutput=False)
    maskT = nc.declare_dram_parameter("maskT", [N, N], BF16, isOutput=False)
    wh_d = nc.declare_dram_parameter("wh", [DIM, 2 * INNER], F16, isOutput=False)
    wl_d = nc.declare_dram_parameter("wl", [DIM, 2 * INNER], F16, isOutput=False)
    wvbf = nc.declare_dram_parameter("wvbf", [DIM, INNER], BF16, isOutput=False)
    wobf = nc.declare_dram_parameter("wobf", [INNER, DIM], BF16, isOutput=False)
    y_out = nc.declare_dram_parameter("y", [N, DIM], F32, isOutput=True)
    sc_out = nc.declare_dram_parameter("score", [2, N], F32, isOutput=True)

    with tile.TileContext(nc) as tc:
        with tc.tile_pool(name="cst", bufs=1) as cst, \
             tc.tile_pool(name="wrk", bufs=2) as wrk, \
             tc.tile_pool(name="wrk4", bufs=2) as wrk4, \
             tc.tile_pool(name="eph", bufs=1) as eph, \
             tc.tile_pool(name="yto", bufs=2) as yto, \
             tc.tile_pool(name="ppA", bufs=1, space="PSUM") as ppA, \
             tc.tile_pool(name="ppB", bufs=1, space="PSUM") as ppB, \
             tc.tile_pool(name="poT", bufs=1, space="PSUM") as poT, \
             tc.tile_pool(name="ppk", bufs=1, space="PSUM") as ppk:

            # ---- loads ----
            xh_t, xl_t, wh_t, wl_t = [], [], [], []
            for kt in range(4):
                r = slice(kt * 128, (kt + 1) * 128)
                a = cst.tile([128, N], F16, tag=f"xh{kt}")
                nc.sync.dma_start(out=a, in_=xh_d[r, :])
                b = cst.tile([128, N], F16, tag=f"xl{kt}")
                nc.sync.dma_start(out=b, in_=xl_d[r, :])
                c = cst.tile([128, 2 * INNER], F16, tag=f"wh{kt}")
                nc.sync.dma_start(out=c, in_=wh_d[r, :])
                d = cst.tile([128, 2 * INNER], F16, tag=f"wl{kt}")
                nc.sync.dma_start(out=d, in_=wl_d[r, :])
                xh_t.append(a)
                xl_t.append(b)
                wh_t.append(c)
                wl_t.append(d)
            xtb = cst.tile([128, 4, N], BF16)
            nc.sync.dma_start(out=xtb, in_=xTbf[:, :].rearrange("(t p) i -> p t i", p=128))
            msk = cst.tile([128, 8, N], BF16)
            nc.sync.dma_start(out=msk, in_=maskT[:, :].rearrange("(t p) i -> p t i", p=128))
            wvb = cst.tile([128, 4, INNER], BF16)
            nc.sync.dma_start(out=wvb, in_=wvbf[:, :].rearrange("(t p) c -> p t c", p=128))
            wob = cst.tile([128, 4, DIM], BF16)
            nc.sync.dma_start(out=wob, in_=wobf[:, :].rearrange("(t p) e -> p t e", p=128))

            ones32 = cst.tile([128, 1], F16)
            nc.vector.memset(ones32, 1.0)
            onesbf = cst.tile([128, 1], BF16)
            nc.vector.memset(onesbf, 1.0)
            onesr1 = cst.tile([1, 128], BF16)
            nc.vector.memset(onesr1, 1.0)

            # hi/lo fp16 q,k: [0:64]=head 2pr, [64:128]=head 2pr+1
            cqm = cst.tile([128, 4, N], F16)
            ckm = cst.tile([128, 4, N], F16)
            ql = cst.tile([128, 4, N], F16)
            kl = cst.tile([128, 4, N], F16)
            vv = cst.tile([128, HEADS, 8, DH], BF16)
            onorm = cst.tile([128, 4, N], BF16)
            sc_acc = cst.tile([128, N], F32)
            nc.vector.memset(sc_acc, 0.0)

            # ---- QK proj (fp16 hi/lo, fp32 accum) + device hi/lo split ----
            for ct in range(8):
                qtag = "dA" if ct % 2 == 0 else "dB"
                qpool = ppA if qtag == "dA" else ppB
                pq = qpool.tile([128, N], F32, tag=qtag)
                cs = slice(ct * 128, (ct + 1) * 128)
                for kt in range(4):
                    nc.tensor.matmul(
                        pq[:, :], wh_t[kt][:, cs], xh_t[kt][:, :],
                        start=(kt == 0), stop=False)
                for kt in range(4):
                    nc.tensor.matmul(
                        pq[:, :], wl_t[kt][:, cs], xh_t[kt][:, :],
                        start=False, stop=False)
                for kt in range(4):
                    nc.tensor.matmul(
                        pq[:, :], wh_t[kt][:, cs], xl_t[kt][:, :],
                        start=False, stop=(kt == 3))
                hi = (cqm if ct < 4 else ckm)[:, ct % 4, :]
                lo = (ql if ct < 4 else kl)[:, ct % 4, :]
                nc.scalar.activation(out=hi, in_=pq[:, :], func=AFT.Copy)
                nc.vector.tensor_tensor(out=lo, in0=pq[:, :], in1=hi,
                                        op=AOP.subtract)

            # ---- V part (bf16) ----
            for jt in range(8):
                vpool, vtag = (ppB, "dB") if jt % 2 == 0 else (ppA, "dA")
                pv = vpool.tile([128, N], F32, tag=vtag)
                for kt in range(4):
                    nc.tensor.matmul(
                        pv[:, 0:512],
                        xtb[:, kt, jt * 128:(jt + 1) * 128],
                        wvb[:, kt, :],
                        start=(kt == 0), stop=(kt == 3))
                nc.vector.tensor_copy(
                    vv[:, :, jt, :],
                    pv[:, 0:512].rearrange("p (h d) -> p h d", h=HEADS))

            # ---- attention, head pairs ----
            pack = ppk.tile([128, N], F32, tag="pk")
            for pr in range(4):
                oTp = poT.tile([128, N], F32, tag="oT")
                carry = None
                for jt in range(8):
                    dA = ppA.tile([128, N], F32, tag="dA")
                    dB = ppB.tile([128, N], F32, tag="dB")
                    jb = slice(jt * 128, (jt + 1) * 128)
                    for (dst, rows, tp) in ((dA, slice(0, 64), (0, 0)),
                                            (dB, slice(64, 128), (64, 0))):
                        nc.tensor.matmul(
                            dst[:, :], ckm[rows, pr, jb], cqm[rows, pr, :],
                            start=True, stop=False, tile_position=tp)
                        nc.tensor.matmul(
                            dst[:, :], kl[rows, pr, jb], cqm[rows, pr, :],
                            start=False, stop=False, tile_position=tp)
                        nc.tensor.matmul(
                            dst[:, :], ckm[rows, pr, jb], ql[rows, pr, :],
                            start=False, stop=True, tile_position=tp)
                    if carry is not None:
                        _emit_burst(nc, oTp, pack, vv, ones32, onesbf, pr, *carry)
                    t = wrk.tile([128, 2, N], F32, tag="t")
                    nc.vector.tensor_tensor(out=t[:, 0, :], in0=dA,
                                            in1=msk[:, jt, :], op=AOP.mult)
                    nc.vector.tensor_tensor(out=t[:, 1, :], in0=dB,
                                            in1=msk[:, jt, :], op=AOP.mult)
                    e2 = wrk4.tile([128, 2, N], BF16, tag="e")
                    nc.scalar.activation(out=e2[:, 0, :], in_=t[:, 0, :],
                                         func=AFT.Exp, scale=SCALE)
                    nc.scalar.activation(out=e2[:, 1, :], in_=t[:, 1, :],
                                         func=AFT.Exp, scale=SCALE)
                    ab2 = wrk4.tile([128, 2, N], F32, tag="ab")
                    nc.vector.tensor_scalar(
                        out=ab2.bitcast(U32), in0=t.bitcast(U32),
                        scalar1=0x7FFFFFFF, scalar2=None, op0=AOP.bitwise_and)
                    es = [e2[:, 0, :], e2[:, 1, :]]
                    abs_ = [ab2[:, 0, :], ab2[:, 1, :]]
                    carry = (jt, es, abs_)
                _emit_burst(nc, oTp, pack, vv, ones32, onesbf, pr, *carry)
                # Z rows: A at row32, B at row96 (fp32 in PSUM)
                zrow = eph.tile([1, 2, N], BF16, tag="zrow")
                nc.scalar.activation(out=zrow[0:1, 0, :], in_=pack[32:33, :],
                                     func=AFT.Copy)
                nc.scalar.activation(out=zrow[0:1, 1, :], in_=pack[96:97, :],
                                     func=AFT.Copy)
                zbc = ppA.tile([128, N], F32, tag="dA")
                nc.tensor.matmul(zbc[0:64, :], onesr1[:, 0:64],
                                 zrow[0:1, 0, :],
                                 start=True, stop=True, tile_position=(0, 0))
                nc.tensor.matmul(zbc[64:128, :], onesr1[:, 0:64],
                                 zrow[0:1, 1, :],
                                 start=True, stop=True, tile_position=(0, 64))
                zr = eph.tile([128, N], F32, tag="zr")
                nc.vector.reciprocal_approx_fast(out=zr, in_=zbc)
                for ic in range(2):
                    sl = slice(ic * 512, (ic + 1) * 512)
                    nc.vector.tensor_tensor(out=onorm[:, pr, sl],
                                            in0=oTp[:, sl], in1=zr[:, sl],
                                            op=AOP.mult)
                nc.vector.tensor_tensor(out=sc_acc[0:97, :], in0=sc_acc[0:97, :],
                                        in1=pack[0:97, :], op=AOP.add)

            # ---- output projection (per pair, K=128), DMA straight from PSUM ----
            for it in range(8):
                ypool, ytag = (ppA, "dA") if it % 2 == 0 else (ppB, "dB")
                yp = ypool.tile([128, N], F32, tag=ytag)
                for pr in range(4):
                    nc.tensor.matmul(
                        yp[:, 0:512],
                        onorm[:, pr, it * 128:(it + 1) * 128],
                        wob[:, pr, :],
                        start=(pr == 0), stop=(pr == 3))
                yt = yto.tile([128, DIM], F32, tag="yt")
                nc.scalar.activation(out=yt, in_=yp[:, 0:512], func=AFT.Copy)
                nc.sync.dma_start(out=y_out[it * 128:(it + 1) * 128, :], in_=yt)

            # ---- raw score row sums (host divides by nnz and scales) ----
            nc.gpsimd.dma_start(out=sc_out[0:1, :], in_=sc_acc[0:1, :])
            nc.gpsimd.dma_start(out=sc_out[1:2, :], in_=sc_acc[64:65, :])
    nc.finalize()
    return nc


def _get_nc():
    if "nc" not in _cache:
        _cache["nc"] = _build()
    return _cache["nc"]


def _f16_split(a):
    hi = a.astype(np.float16)
    lo = (a.astype(np.float32) - hi.astype(np.float32)).astype(np.float16)
    return hi, lo


def _run_device(inputs, trace=False):
    x = np.asarray(inputs["x"], np.float32)
    cp_mask = np.asarray(inputs["cp_mask"])
    w_qkv = np.asarray(inputs["w_qkv"], np.float32)
    w_out = np.asarray(inputs["w_out"], np.float32)

    bf = mybir.dt.np(BF16)
    maskT = np.ascontiguousarray(cp_mask.T).astype(bf)
    wqk = np.ascontiguousarray(w_qkv[:, :2 * INNER])
    wh, wl = _f16_split(wqk)
    wvbf = np.ascontiguousarray(w_qkv[:, 2 * INNER:]).astype(bf)
    wobf = np.ascontiguousarray(w_out).astype(bf)

    in_maps = []
    for b in range(B):
        xTb = np.ascontiguousarray(x[b].T)
        xhh, xll = _f16_split(xTb)
        in_maps.append({
            "xh": xhh,
            "xl": xll,
            "xTbf": xTb.astype(bf),
            "maskT": maskT,
            "wh": wh,
            "wl": wl,
            "wvbf": wvbf,
            "wobf": wobf,
        })

    nc = _get_nc()
    res = run_bass_kernel_spmd(nc, in_maps, core_ids=list(range(B)), trace=trace)
    nnz = np.count_nonzero(cp_mask, axis=1).astype(np.float64)
    b_out = np.asarray(inputs["b_out"], np.float32)
    ys, scores = [], []
    for b in range(B):
        sc = res.results[b]["score"].astype(np.float64)
        scores.append((sc[0] + sc[1]) * SCALE / nnz)
        ys.append(res.results[b]["y"] + b_out[None, :])
    return np.stack(ys).astype(np.float32), np.stack(scores), res


def _apply_swap(y, score, patches):
    idx = np.argsort(score, axis=-1, kind="stable")[::-1]
    out = y.copy()
    clone = y
    bi = np.arange(B)
    for i in range(1, patches + 1):
        ti = idx[:, i]
        out[bi, i] = clone[bi, ti]
        out[bi, ti] = clone[:, i]
    return out


def kernel(**inputs):
    patches = int(np.asarray(inputs["patches_in_core_nodes"]))
    y, score, _ = _run_device(inputs, trace=False)
    return _apply_swap(y, score, patches)


# revision 33
# speedup vs baseline: 1.0052x; 1.0052x over previous
"""CPAttention Trainium2 kernel: 8-way batch-data-parallel over 8 NeuronCores.

v3: fp16 hi/lo split matmuls for the argsort-critical score path.
  - qkproj: x,w split on HOST into fp16 (hi, lo); q = xh*wh + xh*wl + xl*wh
    (12 fp16 matmuls per output tile, fp32 PSUM accumulate, err ~2^-24)
  - dots: q,k split on DEVICE into fp16 hi/lo (Scalar: hi cast, Vector: lo sub);
    d = hiK*hiQ + loK*hiQ + hiK*loQ  (6 K=64 fp16 matmuls per tile, row-packed
    head pairs at tile_position (0,0)/(64,0))
  - score pack rows accumulate across ALL prs in one persistent PSUM tile:
    row0 = score_A, row64 = score_B (fp32); Z_A row32 / Z_B row96 per pr (bf16)
  - nnz, score scaling, bias add, argsort + 16-step row swap: on host
  - output projection DMAs straight from PSUM (no bias add on device)
Softmax/output path is bf16 (rel-err budget 2e-2); score path is fp32-class.
"""
import numpy as np

import concourse.bacc as bacc
import concourse.tile as tile
from concourse import mybir
from concourse.bass_utils import run_bass_kernel_spmd

F32 = mybir.dt.float32
F32R = mybir.dt.float32r
F16 = mybir.dt.float16
BF16 = mybir.dt.bfloat16
U32 = mybir.dt.uint32
AOP = mybir.AluOpType
AFT = mybir.ActivationFunctionType

B, N, DIM = 8, 1024, 512
HEADS, DH = 8, 64
INNER = 512
SCALE = DH ** -0.5

_cache = {}


def _emit_burst(nc, oTp, pack, vv, ones32, onesbf, pr, jt, es, abs_):
    first, last = (jt == 0), (jt == 7)
    for hh in range(2):
        nc.tensor.matmul(
            oTp[hh * 64:(hh + 1) * 64, :],
            vv[:, 2 * pr + hh, jt, :], es[hh][:, :],
            start=first, stop=last,
            tile_position=(0, hh * 64),
            skip_group_check=True)
    for ic in range(2):
        sl = slice(ic * 512, (ic + 1) * 512)
        for hh in range(2):
            st = hh * 64
            nc.tensor.matmul(
                pack[st:st + 1, sl],
                ones32, abs_[hh][:, sl],
                start=first, stop=last,
                tile_position=(0, st),
                skip_group_check=True)
    for hh in range(2):
        st = 32 + hh * 64
        nc.tensor.matmul(
            pack[st:st + 1, :],
            onesbf, es[hh][:, :],
            start=first, stop=last,
            tile_position=(0, st),
            skip_group_check=True)


def _build():
    nc = bacc.Bacc()
    xh_d = nc.declare_dram_parameter("xh", [DIM, N], F16, isOutput=False)
    xl_d = nc.declare_dram_parameter("xl", [DIM, N], F16, isOutput=False)
    xTbf = nc.declare_dram_parameter("xTbf", [DIM, N], BF16, isOutput=False)
    maskT = nc.declare_dram_parameter("maskT", [N, N], BF16, isOutput=False)
    wh_d = nc.declare_dram_parameter("wh", [DIM, 2 * INNER], F16, isOutput=False)
    wl_d = nc.declare_dram_parameter("wl", [DIM, 2 * INNER], F16, isOutput=False)
    wvbf = nc.declare_dram_parameter("wvbf", [DIM, INNER], BF16, isOutput=False)
    wobf = nc.declare_dram_parameter("wobf", [INNER, DIM], BF16, isOutput=False)
    y_out = nc.declare_dram_parameter("y", [N, DIM], F32, isOutput=True)
    sc_out = nc.declare_dram_parameter("score", [2, N], F32, isOutput=True)

    with tile.TileContext(nc) as tc:
        with tc.tile_pool(name="cst", bufs=1) as cst, \
             tc.tile_pool(name="wrk", bufs=2) as wrk, \
             tc.tile_pool(name="wrk4", bufs=2) as wrk4, \
             tc.tile_pool(name="eph", bufs=1) as eph, \
             tc.tile_pool(name="yto", bufs=2) as yto, \
             tc.tile_pool(name="ppA", bufs=1, space="PSUM") as ppA, \
             tc.tile_pool(name="ppB", bufs=1, space="PSUM") as ppB, \
             tc.tile_pool(name="poT", bufs=1, space="PSUM") as poT, \
             tc.tile_pool(name="ppk", bufs=1, space="PSUM") as ppk:

            # ---- loads ----
            xh_t, xl_t, wh_t, wl_t = [], [], [], []
            for kt in range(4):
                r = slice(kt * 128, (kt + 1) * 128)
                a = cst.tile([128, N], F16, tag=f"xh{kt}")
                nc.sync.dma_start(out=a, in_=xh_d[r, :])
                b = cst.tile([128, N], F16, tag=f"xl{kt}")
                nc.sync.dma_start(out=b, in_=xl_d[r, :])
                c = cst.tile([128, 2 * INNER], F16, tag=f"wh{kt}")
                nc.sync.dma_start(out=c, in_=wh_d[r, :])
                d = cst.tile([128, 2 * INNER], F16, tag=f"wl{kt}")
                nc.sync.dma_start(out=d, in_=wl_d[r, :])
                xh_t.append(a)
                xl_t.append(b)
                wh_t.append(c)
                wl_t.append(d)
            xtb = cst.tile([128, 4, N], BF16)
            nc.sync.dma_start(out=xtb, in_=xTbf[:, :].rearrange("(t p) i -> p t i", p=128))
            msk = cst.tile([128, 8, N], BF16)
            nc.sync.dma_start(out=msk, in_=maskT[:, :].rearrange("(t p) i -> p t i", p=128))
            wvb = cst.tile([128, 4, INNER], BF16)
            nc.sync.dma_start(out=wvb, in_=wvbf[:, :].rearrange("(t p) c -> p t c", p=128))
            wob = cst.tile([128, 4, DIM], BF16)
            nc.sync.dma_start(out=wob, in_=wobf[:, :].rearrange("(t p) e -> p t e", p=128))

            ones32 = cst.tile([128, 1], F16)
            nc.vector.memset(ones32, 1.0)
            onesbf = cst.tile([128, 1], BF16)
            nc.vector.memset(onesbf, 1.0)
            onesr1 = cst.tile([1, 128], BF16)
            nc.vector.memset(onesr1, 1.0)

            # hi/lo fp16 q,k: [0:64]=head 2pr, [64:128]=head 2pr+1
            cqm = cst.tile([128, 4, N], F16)
            ckm = cst.tile([128, 4, N], F16)
            ql = cst.tile([128, 4, N], F16)
            kl = cst.tile([128, 4, N], F16)
            vv = cst.tile([128, HEADS, 8, DH], BF16)
            onorm = cst.tile([128, 4, N], BF16)
            sc_acc = cst.tile([128, N], F32)
            nc.vector.memset(sc_acc, 0.0)

            # ---- QK proj (fp16 hi/lo, fp32 accum) + device hi/lo split ----
            for ct in range(8):
                qtag = "dA" if ct % 2 == 0 else "dB"
                qpool = ppA if qtag == "dA" else ppB
                pq = qpool.tile([128, N], F32, tag=qtag)
                cs = slice(ct * 128, (ct + 1) * 128)
                for kt in range(4):
                    nc.tensor.matmul(
                        pq[:, :], wh_t[kt][:, cs], xh_t[kt][:, :],
                        start=(kt == 0), stop=False)
                for kt in range(4):
                    nc.tensor.matmul(
                        pq[:, :], wl_t[kt][:, cs], xh_t[kt][:, :],
                        start=False, stop=False)
                for kt in range(4):
                    nc.tensor.matmul(
                        pq[:, :], wh_t[kt][:, cs], xl_t[kt][:, :],
                        start=False, stop=(kt == 3))
                hi = (cqm if ct < 4 else ckm)[:, ct % 4, :]
                lo = (ql if ct < 4 else kl)[:, ct % 4, :]
                nc.scalar.activation(out=hi, in_=pq[:, :], func=AFT.Copy)
                nc.vector.tensor_tensor(out=lo, in0=pq[:, :], in1=hi,
                                        op=AOP.subtract)

            # ---- V part (bf16) ----
            for jt in range(8):
                vpool, vtag = (ppB, "dB") if jt % 2 == 0 else (ppA, "dA")
                pv = vpool.tile([128, N], F32, tag=vtag)
                for kt in range(4):
                    nc.tensor.matmul(
                        pv[:, 0:512],
                        xtb[:, kt, jt * 128:(jt + 1) * 128],
                        wvb[:, kt, :],
                        start=(kt == 0), stop=(kt == 3))
                nc.vector.tensor_copy(
                    vv[:, :, jt, :],
                    pv[:, 0:512].rearrange("p (h d) -> p h d", h=HEADS))

            # ---- attention, head pairs ----
            pack = ppk.tile([128, N], F32, tag="pk")
            for pr in range(4):
                oTp = poT.tile([128, N], F32, tag="oT")
                carry = None
                for jt in range(8):
                    dA = ppA.tile([128, N], F32, tag="dA")
                    dB = ppB.tile([128, N], F32, tag="dB")
                    jb = slice(jt * 128, (jt + 1) * 128)
                    for (dst, rows, tp) in ((dA, slice(0, 64), (0, 0)),
                                            (dB, slice(64, 128), (64, 0))):
                        nc.tensor.matmul(
                            dst[:, :], ckm[rows, pr, jb], cqm[rows, pr, :],
                            start=True, stop=False, tile_position=tp)
                        nc.tensor.matmul(
                            dst[:, :], kl[rows, pr, jb], cqm[rows, pr, :],
                            start=False, stop=False, tile_position=tp)
                        nc.tensor.matmul(
                            dst[:, :], ckm[rows, pr, jb], ql[rows, pr, :],
                            start=False, stop=True, tile_position=tp)
                    if carry is not None:
                        _emit_burst(nc, oTp, pack, vv, ones32, onesbf, pr, *carry)
                    t = wrk.tile([128, 2, N], F32, tag="t")
                    nc.vector.tensor_tensor(out=t[:, 0, :], in0=dA,
                                            in1=msk[:, jt, :], op=AOP.mult)
                    nc.vector.tensor_tensor(out=t[:, 1, :], in0=dB,
                                            in1=msk[:, jt, :], op=AOP.mult)
                    e2 = wrk4.tile([128, 2, N], BF16, tag="e")
                    nc.scalar.activation(out=e2[:, 0, :], in_=t[:, 0, :],
                                         func=AFT.Exp, scale=SCALE)
                    nc.scalar.activation(out=e2[:, 1, :], in_=t[:, 1, :],
                                         func=AFT.Exp, scale=SCALE)
                    ab2 = wrk4.tile([128, 2, N], F32, tag="ab")
                    nc.vector.tensor_scalar(
                        out=ab2.bitcast(U32), in0=t.bitcast(U32),
                        scalar1=0x7FFFFFFF, scalar2=None, op0=AOP.bitwise_and)
                    es = [e2[:, 0, :], e2[:, 1, :]]
                    abs_ = [ab2[:, 0, :], ab2[:, 1, :]]
                    carry = (jt, es, abs_)
                _emit_burst(nc, oTp, pack, vv, ones32, onesbf, pr, *carry)
                # Z rows: A at row32, B at row96 (fp32 in PSUM)
                zrow = eph.tile([1, 2, N], BF16, tag="zrow")
                nc.scalar.activation(out=zrow[0:1, 0, :], in_=pack[32:33, :],
                                     func=AFT.Copy)
                nc.scalar.activation(out=zrow[0:1, 1, :], in_=pack[96:97, :],
                                     func=AFT.Copy)
                zbc = ppA.tile([128, N], F32, tag="dA")
                nc.tensor.matmul(zbc[0:64, :], onesr1[:, 0:64],
                                 zrow[0:1, 0, :],
                                 start=True, stop=True, tile_position=(0, 0))
                nc.tensor.matmul(zbc[64:128, :], onesr1[:, 0:64],
                                 zrow[0:1, 1, :],
                                 start=True, stop=True, tile_position=(0, 64))
                zr = eph.tile([128, N], F32, tag="zr")
                nc.vector.reciprocal_approx_fast(out=zr, in_=zbc)
                nc.vector.tensor_tensor(out=onorm[:, pr, :], in0=oTp, in1=zr,
                                        op=AOP.mult)
                nc.vector.tensor_tensor(out=sc_acc[0:97, :], in0=sc_acc[0:97, :],
                                        in1=pack[0:97, :], op=AOP.add)

            # ---- output projection (per pair, K=128), DMA straight from PSUM ----
            for it in range(8):
                ypool, ytag = (ppA, "dA") if it % 2 == 0 else (ppB, "dB")
                yp = ypool.tile([128, N], F32, tag=ytag)
                for pr in range(4):
                    nc.tensor.matmul(
                        yp[:, 0:512],
                        onorm[:, pr, it * 128:(it + 1) * 128],
                        wob[:, pr, :],
                        start=(pr == 0), stop=(pr == 3))
                yt = yto.tile([128, DIM], F32, tag="yt")
                nc.scalar.activation(out=yt, in_=yp[:, 0:512], func=AFT.Copy)
                nc.sync.dma_start(out=y_out[it * 128:(it + 1) * 128, :], in_=yt)

            # ---- raw score row sums (host divides by nnz and scales) ----
            nc.gpsimd.dma_start(out=sc_out[0:1, :], in_=sc_acc[0:1, :])
            nc.gpsimd.dma_start(out=sc_out[1:2, :], in_=sc_acc[64:65, :])
    nc.finalize()
    return nc


def _get_nc():
    if "nc" not in _cache:
        _cache["nc"] = _build()
    return _cache["nc"]


def _f16_split(a):
    hi = a.astype(np.float16)
    lo = (a.astype(np.float32) - hi.astype(np.float32)).astype(np.float16)
    return hi, lo


def _run_device(inputs, trace=False):
    x = np.asarray(inputs["x"], np.float32)
    cp_mask = np.asarray(inputs["cp_mask"])
    w_qkv = np.asarray(inputs["w_qkv"], np.float32)
    w_out = np.asarray(inputs["w_out"], np.float32)

    bf = mybir.dt.np(BF16)
    maskT = np.ascontiguousarray(cp_mask.T).astype(bf)
    wqk = np.ascontiguousarray(w_qkv[:, :2 * INNER])
    wh, wl = _f16_split(wqk)
    wvbf = np.ascontiguousarray(w_qkv[:, 2 * INNER:]).astype(bf)
    wobf = np.ascontiguousarray(w_out).astype(bf)

    in_maps = []
    for b in range(B):
        xTb = np.ascontiguousarray(x[b].T)
        xhh, xll = _f16_split(xTb)
        in_maps.append({
            "xh": xhh,
            "xl": xll,
            "xTbf": xTb.astype(bf),
            "maskT": maskT,
            "wh": wh,
            "wl": wl,
            "wvbf": wvbf,
            "wobf": wobf,
        })

    nc = _get_nc()
    res = run_bass_kernel_spmd(nc, in_maps, core_ids=list(range(B)), trace=trace)
    nnz = np.count_nonzero(cp_mask, axis=1).astype(np.float64)
    b_out = np.asarray(inputs["b_out"], np.float32)
    ys, scores = [], []
    for b in range(B):
        sc = res.results[b]["score"].astype(np.float64)
        scores.append((sc[0] + sc[1]) * SCALE / nnz)
        ys.append(res.results[b]["y"] + b_out[None, :])
    return np.stack(ys).astype(np.float32), np.stack(scores), res


def _apply_swap(y, score, patches):
    idx = np.argsort(score, axis=-1, kind="stable")[::-1]
    out = y.copy()
    clone = y
    bi = np.arange(B)
    for i in range(1, patches + 1):
        ti = idx[:, i]
        out[bi, i] = clone[bi, ti]
        out[bi, ti] = clone[:, i]
    return out


def kernel(**inputs):
    patches = int(np.asarray(inputs["patches_in_core_nodes"]))
    y, score, _ = _run_device(inputs, trace=False)
    return _apply_swap(y, score, patches)
